# revision 33
# baseline (speedup 1.0000x reference)
"""Trainium2 Bass kernel for a 3-layer complex RBF network (v6, 202us).

Math per layer (complex y, G; real phi):
    dist_i = sum_j |y_j - G_ij|^2;  phi = exp(-dist/(2s));  y = W @ phi + b

Distribution (8 cores): hidden axis I=4096 sharded 512/core; per-layer
partial y is AllReduce-summed; b added once post-AR.

Design (evolution 445us -> 343 -> 285 -> 227 -> 202us):
  - Distance expansion dist = sum|y|^2 + sum|G|^2 - 2*sum(yr*Gr + yi*Gi):
    all heavy math runs on the otherwise-idle PE as matmuls; sum|G|^2 and
    1/s terms are host-precomputed aux constants, so the ACT/DVE dist
    squares and matvec reductions of the v2 design vanish.
  - Weights stream as fp8e4 (host x64-prescaled; unscaled via aux and a
    64-valued ones tile for the ynorm fold, and a /64 on the matvec evac).
    19MB/core fully SBUF-resident -> no WAR throttling, AllReduces run
    nearly uncontended.  PE matmuls mix bf16 stationary x fp8 moving.
  - Weight stream rides the sync HWDGE ring: collective_compute blocks its
    issuing queue until the collective COMPLETES, so the gpsimd ring keeps
    only the AR triggers + post-AR cast-DMAs and the stream flows from t=0.
  - Each complex half of y is AllReduced separately (l=1,2) so the next
    layer's r=0 cross matmuls overlap the r=1 reduce.
  - NO DMA-transposes anywhere: Tile serializes those against ALL
    collectives.  phi [1,512]->[128,4] and post-AR y [1,Ol]->[128,32] both
    transpose via K=1 PE matmuls against a [1,1] ones tile into PSUM.
  - x is host-permuted straight into the [128p, (r c)] y-tile layout;
    b3 is seeded into `out` early and AR3 accumulates on top (SWDGE
    accum-DMA); b1/b2 fold into the post-AR y evac on DVE.
"""

import numpy as np

P = 128
NCORES = 8
HID = 4096
IS = HID // NCORES          # 512: per-core shard of the hidden axis
NCH = IS // P               # 4 i-chunks of 128 (i = c*128 + p)
# (Oprev, Ol) for layers 1..3
DIMS = [(1024, 4096), (4096, 4096), (4096, 1024)]
JG = 8                      # j-chunks per G^T slab
SLABW = 2048                # o-columns per W^T slab

_cache = {}


def _geom(l):
    Op, Ol = DIMS[l - 1]
    n_jg = Op // (P * JG)           # G^T slabs per r: 1 / 4 / 4
    slabw = min(Ol, SLABW)
    n_oh = Ol // slabw              # W^T slabs per r: 2 / 2 / 1
    return Op, Ol, n_jg, slabw, n_oh


def _build_nc():
    import concourse.bacc as bacc
    import concourse.mybir as mybir
    import concourse.tile as tile

    f32 = mybir.dt.float32
    bf16 = mybir.dt.bfloat16
    f8 = mybir.dt.float8e4
    AF = mybir.ActivationFunctionType
    ALU = mybir.AluOpType

    nc = bacc.Bacc(None)

    xp = nc.dram_tensor("xp", [P, 16], bf16, kind="ExternalInput")
    GTD, WTD, AUX, BPD = {}, {}, {}, {}
    for l in (1, 2, 3):
        Op, Ol, n_jg, slabw, n_oh = _geom(l)
        GTD[l] = nc.dram_tensor(f"gt{l}", [2, n_jg, P, JG, IS], f8,
                                kind="ExternalInput")
        WTD[l] = nc.dram_tensor(f"wt{l}", [2, n_oh, P, NCH, slabw], f8,
                                kind="ExternalInput")
        AUX[l] = nc.dram_tensor(f"aux{l}", [P, 2 * NCH], f32,
                                kind="ExternalInput")
    for l in (1, 2):
        Ol = DIMS[l - 1][1]
        BPD[l] = nc.dram_tensor(f"bp{l}", [P, 2 * (Ol // P)], bf16,
                                kind="ExternalInput")
    b3f = nc.dram_tensor("b3f", [2, 1024], f32, kind="ExternalInput")
    out = nc.dram_tensor("out", [2, 1024], f32, kind="ExternalOutput")

    RG = [list(range(NCORES))]

    with tile.TileContext(nc) as tc:
        with (
            tc.tile_pool(name="gt", bufs=18) as gtp,     # [128, 8, 512] fp8
            tc.tile_pool(name="wt", bufs=10) as wtp,     # [128, 4, 2048] fp8
            tc.tile_pool(name="yt", bufs=4) as ytp,      # [128, 32] bf16
            tc.tile_pool(name="ysbp", bufs=2) as ysbp,   # [1, 4096] f32
            tc.tile_pool(name="small", bufs=1) as small,
            tc.tile_pool(name="psum", bufs=1, space="PSUM") as psp,
            tc.tile_pool(name="psum_y", bufs=4, space="PSUM") as pyp,
            tc.tile_pool(name="dram", bufs=1, space="DRAM") as dramp,
        ):
            # ---- small constants (scalar HWDGE ring + DVE memsets) ------
            ones512 = small.tile([P, IS], bf16, tag="ones512")
            nc.vector.memset(ones512[:], 64.0)
            one1 = small.tile([1, 1], f32, tag="one1")
            nc.vector.memset(one1[:], 1.0)
            one1b = small.tile([1, 1], bf16, tag="one1b")
            nc.vector.memset(one1b[:], 1.0)
            auxt, bpt = {}, {}
            for l in (1, 2, 3):
                a = small.tile([P, 2 * NCH], f32, tag=f"aux_{l}")
                nc.scalar.dma_start(a[:], AUX[l][:])
                auxt[l] = a
            for l in (1, 2):
                Ol = DIMS[l - 1][1]
                b = small.tile([P, 2 * (Ol // P)], bf16, tag=f"bp_{l}")
                nc.scalar.dma_start(b[:], BPD[l][:])
                bpt[l] = b

            # ---- layer-1 y tiles (one per complex half): host-permuted --
            y1 = {}
            for r in range(2):
                t = ytp.tile([P, 8], bf16, tag="yt")
                nc.scalar.dma_start(t[:], xp[:, r * 8 : (r + 1) * 8])
                y1[r] = t

            # seed out with b3 early; AR3's result accumulates on top later
            nc.gpsimd.dma_start(out[:], b3f[:])

            # ---- weight stream: sync HWDGE ring (no CC instructions on
            # this ring, so it flows from t~=0; later tiles throttle on
            # pool-slot WAR waits only) -----------------------------------
            gtt, wtt = {}, {}

            def emit_gt(l):
                n_jg = _geom(l)[2]
                for r in range(2):
                    for jg in range(n_jg):
                        g = gtp.tile([P, JG, IS], f8, tag="gt")
                        nc.sync.dma_start(g[:], GTD[l][r, jg])
                        gtt[(l, r, jg)] = g

            def emit_wt(l):
                _, _, _, slabw, n_oh = _geom(l)
                for r in range(2):
                    for oh in range(n_oh):
                        w = wtp.tile([P, NCH, slabw], f8, tag="wt")
                        nc.sync.dma_start(w[:], WTD[l][r, oh])
                        wtt[(l, r, oh)] = w

            emit_gt(1)
            emit_wt(1)
            emit_gt(2)
            emit_wt(2)
            emit_gt(3)
            emit_wt(3)

            # ---- per-layer compute --------------------------------------
            ytile = y1
            for l in (1, 2, 3):
                Op, Ol, n_jg, slabw, n_oh = _geom(l)
                C = Op // P             # j-chunks: 8 / 32 / 32
                NT = Ol // 512

                # cross psum group: q[i] = sum_j y_j G_ij  -  ynorm/2
                # (per-r sub-groups so the r=0 half can run while the r=1
                # half of the previous AllReduce is still in flight; each
                # sub-group leads with its own -sum(y_r^2)/2 fold-in matmul)
                crossp = psp.tile([1, IS], f32, tag="cross")
                for r in range(2):
                    yr = ytile[r]
                    ysq = small.tile([P, C], f32, tag="ysq")
                    nc.vector.tensor_mul(ysq[:], yr[:], yr[:])
                    ysnf = small.tile([P, 1], f32, tag="ysnf")
                    nc.vector.tensor_scalar(
                        ysq[:], ysq[:], -0.5, 0.0, ALU.mult, ALU.add,
                        accum_out=ysnf[:],
                    )
                    ysn = small.tile([P, 1], bf16, tag="ysn")
                    nc.vector.tensor_copy(ysn[:], ysnf[:])
                    nc.tensor.matmul(
                        crossp[:], ysn[:], ones512[:],
                        start=(r == 0), stop=False,
                    )
                    for jc in range(C):
                        g = gtt[(l, r, jc // JG)]
                        nc.tensor.matmul(
                            crossp[:], yr[:, jc : jc + 1], g[:, jc % JG, :],
                            start=False,
                            stop=(r == 1 and jc == C - 1),
                        )
                crossS = small.tile([1, IS], f32, tag="crossS")
                nc.vector.tensor_copy(crossS[:], crossp[:])

                # transpose q to [128p, 4c] via 4 K=1 matmuls
                ct = psp.tile([P, NCH], f32, tag="ct")
                for c in range(NCH):
                    nc.tensor.matmul(
                        ct[:, c : c + 1], crossS[0:1, c * P : (c + 1) * P],
                        one1[:], start=True, stop=True,
                    )

                # expin = q/s - gq/(2s);  phi = exp(expin)
                tcomb = small.tile([P, NCH], f32, tag="tcomb")
                nc.vector.tensor_mul(tcomb[:], ct[:], auxt[l][:, 0:NCH])
                nc.vector.tensor_add(
                    tcomb[:], tcomb[:], auxt[l][:, NCH : 2 * NCH]
                )
                phi4 = small.tile([P, NCH], bf16, tag="phi4")
                nc.scalar.activation(phi4[:], tcomb[:], AF.Exp)

                # matvec: y_partial[r, o] = sum_i W[r, o, i] phi_i
                # Each complex half r is staged and AllReduced separately
                # (l<3) so next layer's r=0 cross work overlaps the r=1
                # reduce; layer 3's payload is small -- keep it whole.
                ysbt, ynext = {}, {}
                for r in range(2):
                    h = r if l < 3 else None
                    ysb = ysbt.get(h)
                    if ysb is None:
                        n = Ol if l < 3 else 2 * Ol
                        ysb = ysbp.tile([1, n], f32, tag="ysb", name="ysb")
                        ysbt[h] = ysb
                    for nt in range(NT):
                        oh = (nt * 512) // slabw
                        off = nt * 512 - oh * slabw
                        w = wtt[(l, r, oh)]
                        py = pyp.tile([1, 512], f32, tag="py")
                        for c in range(NCH):
                            nc.tensor.matmul(
                                py[:], phi4[:, c : c + 1],
                                w[:, c, off : off + 512],
                                start=(c == 0), stop=(c == NCH - 1),
                            )
                        col = ((r * NT + nt) * 512) if l == 3 else nt * 512
                        nc.vector.tensor_scalar_mul(
                            ysb[0:1, col : col + 512], py[:], 1.0 / 64
                        )
                    if l < 3 or r == 1:
                        n = Ol if l < 3 else 2 * Ol
                        ccp = dramp.tile([1, n], f32, tag=f"ccp_{l}_{r}")
                        ccq = dramp.tile([1, n], f32, tag=f"ccq_{l}_{r}")
                        nc.scalar.dma_start(ccp[:], ysb[:])
                        nc.gpsimd.collective_compute(
                            "AllReduce", ALU.add, replica_groups=RG,
                            ins=[ccp.opt()], outs=[ccq.opt()],
                        )
                        if l < 3:
                            # per-half post-AR: cast-DMA to a bf16 row, then
                            # PE K=1 transpose matmuls into a psum bank and
                            # one fused bias-add evac.  (No DMA-transpose:
                            # Tile serializes those against ALL collectives,
                            # which would stall this half on the other
                            # half's AllReduce.)
                            yrow = small.tile([1, Ol], bf16, tag=f"yrow_{r}")
                            nc.gpsimd.dma_start(yrow[:], ccq[:])
                            Cn = Ol // P
                            ypb = psp.tile([P, Cn], f32, tag="ypb")
                            for c in range(Cn):
                                nc.tensor.matmul(
                                    ypb[:, c : c + 1],
                                    yrow[0:1, c * P : (c + 1) * P],
                                    one1b[:], start=True, stop=True,
                                )
                            ytr = ytp.tile([P, Cn], bf16, tag="yt")
                            nc.vector.tensor_add(
                                ytr[:], ypb[:],
                                bpt[l][:, r * Cn : (r + 1) * Cn],
                            )
                            ynext[r] = ytr

                if l < 3:
                    ytile = dict(ynext)
                else:
                    # out = b3 (seeded earlier) + AR3 result
                    nc.gpsimd.dma_start(
                        out[:], ccq[:].rearrange("q (r o) -> (q r) o", r=2),
                        accum_op=ALU.add,
                    )

    nc.finalize()
    return nc


def _get_nc():
    if "nc" not in _cache:
        _cache["nc"] = _build_nc()
    return _cache["nc"]


def make_in_maps(inputs):
    """Host-side sharding + layout prep (bf16 casts, transposed weight tile
    layouts, weight-derived aux constants, permuted biases)."""
    import ml_dtypes

    bf = ml_dtypes.bfloat16
    f8 = ml_dtypes.float8_e4m3
    x = np.asarray(inputs["x"], dtype=np.float32)
    # xp[p, r*8 + c] = x[r, c*128 + p]  (the [128p, (r c)] y-tile layout)
    xp = np.ascontiguousarray(
        np.transpose(x.reshape(2, 8, P), (2, 0, 1)).reshape(P, 16).astype(bf)
    )
    b3 = np.ascontiguousarray(inputs["b3"], dtype=np.float32)

    in_maps = []
    for core in range(NCORES):
        lo, hi = core * IS, (core + 1) * IS
        m = {"xp": xp, "b3f": b3}
        for l in (1, 2, 3):
            Op, Ol, n_jg, slabw, n_oh = _geom(l)
            G = np.asarray(inputs[f"G{l}"][:, lo:hi, :], dtype=np.float32)
            W = np.asarray(inputs[f"W{l}"][:, :, lo:hi], dtype=np.float32)
            s = np.asarray(inputs[f"s{l}"][lo:hi], dtype=np.float32)
            # gth[r, jg, p, q, i] = G[r, lo+i, jg*1024 + q*128 + p]
            gth = np.transpose(
                G.reshape(2, IS, n_jg, JG, P), (0, 2, 4, 3, 1)
            )
            m[f"gt{l}"] = np.ascontiguousarray((gth * 64.0).astype(f8))
            # wth[r, oh, p, c, o'] = W[r, oh*slabw+o', lo + c*128 + p]
            wth = np.transpose(
                W.reshape(2, n_oh, slabw, NCH, P), (0, 1, 4, 3, 2)
            )
            m[f"wt{l}"] = np.ascontiguousarray((wth * 64.0).astype(f8))
            # aux: cols 0:4 -> 1/s ; cols 4:8 -> -sum|G|^2/(2s)   (i = c*128+p)
            gq = (G[0] ** 2 + G[1] ** 2).sum(axis=-1)       # [IS]
            a = np.empty((P, 2 * NCH), dtype=np.float32)
            a[:, 0:NCH] = (1.0 / (64.0 * s)).reshape(NCH, P).T
            a[:, NCH:] = (gq * (-0.5 / s)).reshape(NCH, P).T
            m[f"aux{l}"] = a
            if l < 3:
                b = np.asarray(inputs[f"b{l}"], dtype=np.float32)
                # bp[p, r*C + c] = b[r, c*128 + p]
                Cl = Ol // P
                bp = np.transpose(b.reshape(2, Cl, P), (2, 0, 1)).reshape(
                    P, 2 * Cl
                )
                m[f"bp{l}"] = np.ascontiguousarray(bp.astype(bf))
        in_maps.append(m)
    return in_maps


def run(inputs, trace=False, **kw):
    from concourse.bass_utils import run_bass_kernel_spmd

    nc = _get_nc()
    in_maps = make_in_maps(inputs)
    res = run_bass_kernel_spmd(nc, in_maps, list(range(NCORES)), trace=trace, **kw)
    return res


def kernel(**inputs):
    res = run(inputs, trace=False)
    return np.asarray(res.results[0]["out"], dtype=np.float32)
